# revision 1
# baseline (speedup 1.0000x reference)
"""Trainium2 Bass kernel for nn_MACBlock (segmented attention + GEGLU FFN).

Sharding: 8 cores = 2 batches x 4 segments of 512 queries. The segment mask
makes attention block-diagonal (plus a 32-token always-visible prefix derived
from pooled memory + persistent memory), so each core is fully independent:
no collectives.

Layout: activations are kept feature-major (x^T [dim, tokens]) on-chip, so
every matmul contraction dim lands on partitions with zero transposes.
Scores are computed key-major ([keys, queries]); softmax is max-free (scores
are small by construction); the softmax denominator comes from an all-ones
stationary operand accumulated into the same PSUM tile as P@V.
Matmuls run as float32r (full fp32 data, full PE rate at free-dim>=256).
"""

import sys

if "/opt/trn_rl_repo" not in sys.path:
    sys.path.insert(0, "/opt/trn_rl_repo")

import numpy as np

B, N, DIM = 2, 2048, 1024
HEADS, DH = 16, 64
SEG = 512
NPM = NM = 16
PFX = NPM + NM          # 32 prefix keys
DFF = 2730
MFF = 22                # padded dff chunks
DFFP = MFF * 128        # 2816
KO = 8                  # 1024 / 128
P = 128
NCORES = 8
EPS = 1.1920929e-07
NEG = -1.0e9

_CACHE = {}


def _f32r(ap):
    import concourse.mybir as mybir
    return ap.bitcast(mybir.dt.float32r)


def build_nc(reps=1):
    import concourse.bass as bass
    from concourse import bacc
    import concourse.tile as tile
    import concourse.mybir as mybir

    f32 = mybir.dt.float32
    AF = mybir.ActivationFunctionType
    OP = mybir.AluOpType
    AX = mybir.AxisListType

    nc = bacc.Bacc("TRN2", target_bir_lowering=False, debug=False)

    dp = nc.declare_dram_parameter
    xT_d = dp("xT", [DIM, SEG], f32, isOutput=False)
    mo_d = dp("mo", [N, DIM], f32, isOutput=False)
    cq_d = dp("cq", [P, SEG], f32, isOutput=False)
    sq_d = dp("sq", [P, SEG], f32, isOutput=False)
    ck_d = dp("ck", [P, SEG], f32, isOutput=False)
    sk_d = dp("sk", [P, SEG], f32, isOutput=False)
    mask_d = dp("maskD", [P, P], f32, isOutput=False)
    rmat_d = dp("rmat", [P, P], f32, isOutput=False)
    ones_d = dp("ones", [P, P], f32, isOutput=False)
    qkw_d = dp("qkw", [16, P, KO, P], f32, isOutput=False)
    kvw_d = dp("kvw", [2, KO, P, DIM], f32, isOutput=False)
    outw_d = dp("outw", [KO, P, KO, P], f32, isOutput=False)
    w1a_d = dp("w1a", [MFF, P, KO, P], f32, isOutput=False)
    w1g_d = dp("w1g", [MFF, P, KO, P], f32, isOutput=False)
    w2_d = dp("w2", [KO, P, MFF, P], f32, isOutput=False)
    mtw_d = dp("mtw", [KO, P, DIM], f32, isOutput=False)
    pmv_d = dp("pmv", [HEADS, NPM, DH], f32, isOutput=False)
    pmk_d = dp("pmk", [HEADS, DH, NPM], f32, isOutput=False)
    b1a_d = dp("b1a", [P, MFF], f32, isOutput=False)
    b1g_d = dp("b1g", [P, MFF], f32, isOutput=False)
    b2_d = dp("b2", [P, KO], f32, isOutput=False)
    anw_d = dp("anw", [P, KO], f32, isOutput=False)
    fnw_d = dp("fnw", [P, KO], f32, isOutput=False)
    mpnw_d = dp("mpnw", [1, DIM], f32, isOutput=False)
    yT_d = dp("yT", [DIM, SEG], f32, isOutput=True)

    def _emit(nc):
      with tile.TileContext(nc) as tc, \
            nc.allow_low_precision(reason="float32r matmul rounding"):
        from contextlib import ExitStack
        ctx = ExitStack()
        with ctx:
            persist = ctx.enter_context(tc.tile_pool(name="persist", bufs=1))
            wpool = ctx.enter_context(tc.tile_pool(name="wpool", bufs=3))
            kvpool = ctx.enter_context(tc.tile_pool(name="kvpool", bufs=2))
            w2pool = ctx.enter_context(tc.tile_pool(name="w2pool", bufs=2))
            mopool = ctx.enter_context(tc.tile_pool(name="mopool", bufs=2))
            rot = ctx.enter_context(tc.tile_pool(name="rot", bufs=2))
            epool = ctx.enter_context(tc.tile_pool(name="epool", bufs=2))
            pa = ctx.enter_context(tc.tile_pool(name="pa", bufs=4, space="PSUM"))
            psc = ctx.enter_context(tc.tile_pool(name="psc", bufs=2, space="PSUM"))
            pso = ctx.enter_context(tc.tile_pool(name="pso", bufs=2, space="PSUM"))

            cnt = [0]

            def pa_t():
                cnt[0] += 1
                return pa.tile([P, SEG], f32, tag="ps", name=f"pa{cnt[0]}")

            def psc_t():
                cnt[0] += 1
                return psc.tile([P, SEG], f32, tag="sc", name=f"sc{cnt[0]}")

            def pso_t():
                cnt[0] += 1
                return pso.tile([P, SEG], f32, tag="o", name=f"o{cnt[0]}")

            # ---------------- persistent SBUF tensors ----------------
            xT = persist.tile([P, KO, SEG], f32, tag="xT")       # x^T, later x1^T
            xnT = persist.tile([P, KO, SEG], f32, tag="xnT")     # xn^T, later xn1^T
            kT = persist.tile([P, KO, SEG], f32, tag="kT")       # roped k^T
            vA = persist.tile([P, 4, HEADS, DH], f32, tag="vA")  # v key-major
            vP = persist.tile([PFX, HEADS, DH], f32, tag="vP")   # prefix v rows
            kP = persist.tile([P, HEADS, PFX], f32, tag="kP")    # prefix k^T @64*(h%2)
            oA = persist.tile([P, KO, SEG], f32, tag="oA")       # attn o^T, later outT
            cq = persist.tile([P, SEG], f32, tag="cq")
            sq_ = persist.tile([P, SEG], f32, tag="sq")
            ck = persist.tile([P, SEG], f32, tag="ck")
            sk = persist.tile([P, SEG], f32, tag="sk")
            maskD = persist.tile([P, P], f32, tag="maskD")
            rmat = persist.tile([P, P], f32, tag="rmat")
            b1a = persist.tile([P, MFF], f32, tag="b1a")
            b1g = persist.tile([P, MFF], f32, tag="b1g")
            b2 = persist.tile([P, KO], f32, tag="b2")
            anw = persist.tile([P, KO], f32, tag="anw")
            fnw = persist.tile([P, KO], f32, tag="fnw")
            mpnw = persist.tile([1, DIM], f32, tag="mpnw")
            ones128 = persist.tile([P, 1], f32, tag="o128")      # lhsT K=128,M=1
            ones1x128 = persist.tile([1, P], f32, tag="o1x128")  # lhsT K=1,M=128
            ones16 = persist.tile([1, 16], f32, tag="o16")
            ones11 = persist.tile([1, 1], f32, tag="o11")
            onesPV = persist.tile([P, DH], f32, tag="oPV")       # sums stationary
            pooledT = persist.tile([P, KO], f32, tag="pooledT")
            memtokT = persist.tile([P, KO], f32, tag="memtokT")
            mrow = persist.tile([1, 3 * DIM], f32, tag="mrow")
            rrow = persist.tile([1, DIM], f32, tag="rrow")
            epsc = persist.tile([P, 1], f32, tag="epsc")
            zeroc = persist.tile([P, 1], f32, tag="zeroc")

            dma = nc.sync.dma_start
            dma(out=cq, in_=cq_d[:])
            dma(out=sq_, in_=sq_d[:])
            dma(out=ck, in_=ck_d[:])
            dma(out=sk, in_=sk_d[:])
            dma(out=maskD, in_=mask_d[:])
            dma(out=_f32r(rmat), in_=_f32r(rmat_d[:]))
            dma(out=b1a, in_=b1a_d[:])
            dma(out=b1g, in_=b1g_d[:])
            dma(out=b2, in_=b2_d[:])
            dma(out=anw, in_=anw_d[:])
            dma(out=fnw, in_=fnw_d[:])
            dma(out=mpnw, in_=mpnw_d[:])
            dma(out=_f32r(ones128), in_=_f32r(ones_d[:, 0:1]))
            dma(out=_f32r(ones1x128), in_=_f32r(ones_d[0:1, :]))
            dma(out=_f32r(ones16), in_=_f32r(ones_d[0:1, 0:16]))
            dma(out=_f32r(ones11), in_=_f32r(ones_d[0:1, 0:1]))
            dma(out=_f32r(onesPV), in_=_f32r(ones_d[:, 0:DH]))
            nc.vector.memset(epsc, EPS)
            nc.vector.memset(zeroc, 0.0)
            for h in range(HEADS):
                hb = DH * (h % 2)
                dma(out=_f32r(kP[hb:hb + DH, h, NPM:PFX]), in_=_f32r(pmk_d[h]))
                dma(out=_f32r(vP[NPM:PFX, h, :]), in_=_f32r(pmv_d[h]))

            if True:
              dma(out=xT, in_=xT_d.rearrange("(ko p) n -> p ko n", p=P))

              mm = nc.tensor.matmul

              def rmsnorm_into(dst, src, w_sb, sq_tag):
                  """dst[:,ko,:] = src[:,ko,:] * w[:,ko] * rsqrt(mean_dim(src^2)+eps)"""
                  ss = psc_t()  # [1,512] slice used
                  sq8 = persist.tile([P, KO, SEG], f32, tag=sq_tag, name="sq8")
                  for ko in range(KO):
                      nc.vector.tensor_mul(_f32r(sq8[:, ko, :]), src[:, ko, :],
                                           src[:, ko, :])
                      mm(ss[0:1, :], _f32r(ones128), _f32r(sq8[:, ko, :]),
                         start=(ko == 0), stop=(ko == KO - 1))
                  rr = rrow
                  nc.scalar.activation(_f32r(rr[:, 0:SEG]), ss[0:1, :], AF.Sqrt,
                                       bias=epsc[0:1], scale=1.0 / DIM)
                  nc.vector.reciprocal(_f32r(rr[:, SEG:2 * SEG]), rr[:, 0:SEG])
                  bc = pso_t()  # broadcast rstd over 128 partitions
                  mm(bc, ones1x128, rr[:, SEG:2 * SEG],
                     start=True, stop=True)
                  for ko in range(KO):
                      nc.vector.scalar_tensor_tensor(
                          out=_f32r(dst[:, ko, :]), in0=src[:, ko, :],
                          scalar=w_sb[:, ko:ko + 1], in1=bc,
                          op0=OP.mult, op1=OP.mult)

              # ---------------- attn rmsnorm ----------------
              rmsnorm_into(xnT, xT, anw, "big16")
              qT = persist.tile([P, KO, SEG], f32, tag="qT")       # roped,scaled q^T

              # ---------------- q/k projections + rope, interleaved with
              # ---------------- mem_out mean accumulation ----------------
              mean_ps = [psc_t(), psc_t()]   # two [1,512] accumulators (slices)

              def mo_mean_step(t):
                  mot = mopool.tile([P, DIM], f32, tag="mo", name="mot")
                  dma(out=_f32r(mot), in_=_f32r(mo_d[t * P:(t + 1) * P, :]))
                  for half in range(2):
                      mm(mean_ps[half][0:1, :], _f32r(ones128),
                         _f32r(mot[:, half * SEG:(half + 1) * SEG]),
                         start=(t == 0), stop=(t == 15))

              for m in range(16):
                  wt = wpool.tile([P, KO, P], f32, tag="w8")
                  dma(out=_f32r(wt), in_=_f32r(qkw_d[m]))
                  ps = pa_t()
                  for ko in range(KO):
                      mm(ps, _f32r(wt[:, ko]), _f32r(xnT[:, ko, :]),
                         start=(ko == 0), stop=(ko == KO - 1))
                  is_q = m < 8
                  c_t, s_t = (cq, sq_) if is_q else (ck, sk)
                  dst = qT if is_q else kT
                  ko_out = m % 8
                  qraw = rot.tile([P, SEG], f32, tag="ropeA")
                  nc.scalar.copy(_f32r(qraw), ps)
                  rps = pa_t()
                  mm(rps, _f32r(rmat), _f32r(qraw), start=True, stop=True)
                  At = rot.tile([P, SEG], f32, tag="ropeB")
                  nc.vector.tensor_mul(At, ps, c_t)
                  Bt = rot.tile([P, SEG], f32, tag="ropeA")
                  nc.vector.tensor_mul(Bt, rps, s_t)
                  nc.vector.tensor_add(_f32r(dst[:, ko_out, :]), At, Bt)
                  mo_mean_step(m)

              # ---------------- v projection (token-major) ----------------
              for half in range(2):
                  kvv = persist.tile([P, KO, SEG], f32, tag="big16")
                  for ko in range(KO):
                      dma(out=_f32r(kvv[:, ko, :]),
                          in_=_f32r(kvw_d[1, ko, :, half * SEG:(half + 1) * SEG]))
                  for tc_ in range(4):
                      ps = pa_t()
                      for ko in range(KO):
                          mm(ps, _f32r(xnT[:, ko, tc_ * P:(tc_ + 1) * P]),
                             _f32r(kvv[:, ko, :]),
                             start=(ko == 0), stop=(ko == KO - 1))
                      nc.vector.tensor_copy(
                          out=_f32r(vA[:, tc_, half * 8:(half + 1) * 8, :]),
                          in_=ps.rearrange("p (h d) -> p h d", d=DH))

              # ---------------- memory-context chain ----------------
              pooled_raw = mrow[:, 0:DIM]
              for half in range(2):
                  nc.scalar.activation(_f32r(pooled_raw[:, half * SEG:(half + 1) * SEG]),
                                       mean_ps[half][0:1, :], AF.Copy,
                                       scale=1.0 / N)
              sqr = mrow[:, DIM:2 * DIM]
              nc.vector.tensor_mul(_f32r(sqr), pooled_raw, pooled_raw)
              nc.vector.reduce_sum(_f32r(sqr[:, 0:1]), sqr, axis=AX.X)
              nc.scalar.activation(_f32r(sqr[:, 1:2]), sqr[:, 0:1], AF.Sqrt,
                                   bias=epsc[0:1], scale=1.0 / DIM)
              nc.vector.reciprocal(_f32r(sqr[:, 2:3]), sqr[:, 1:2])
              pooled = mrow[:, 2 * DIM:3 * DIM]
              nc.vector.scalar_tensor_tensor(out=_f32r(pooled), in0=pooled_raw,
                                             scalar=sqr[:, 2:3], in1=mpnw,
                                             op0=OP.mult, op1=OP.mult)
              # pooled^T via K=1 transpose matmuls
              pT = pa_t()
              for ko in range(KO):
                  mm(pT[:, ko:ko + 1], pooled[0:1, ko * P:(ko + 1) * P],
                     ones11, start=True, stop=True, skip_group_check=True)
              nc.vector.tensor_copy(out=_f32r(pooledT), in_=pT[:, 0:KO])
              # mem_tok row = pooled @ to_mem_tokens_w
              mt_ps = [psc_t(), psc_t()]
              for ko in range(KO):
                  mtw_t = kvpool.tile([P, DIM], f32, tag="kv")
                  dma(out=_f32r(mtw_t), in_=_f32r(mtw_d[ko]))
                  for half in range(2):
                      mm(mt_ps[half][0:1, :], _f32r(pooledT[:, ko:ko + 1]),
                         _f32r(mtw_t[:, half * SEG:(half + 1) * SEG]),
                         start=(ko == 0), stop=(ko == KO - 1))
              memtok = mrow[:, 0:DIM]
              for half in range(2):
                  nc.scalar.activation(_f32r(memtok[:, half * SEG:(half + 1) * SEG]),
                                       mt_ps[half][0:1, :], AF.Copy)
              mT = pa_t()
              for ko in range(KO):
                  mm(mT[:, ko:ko + 1], memtok[0:1, ko * P:(ko + 1) * P],
                     ones11, start=True, stop=True, skip_group_check=True)
              nc.vector.tensor_copy(out=_f32r(memtokT), in_=mT[:, 0:KO])
              # k_c / v_c rows = mem_tok @ Wk / Wv
              kcvc = []
              for c in range(2):
                  r_ps = [psc_t(), psc_t()]
                  for ko in range(KO):
                      kv_t = kvpool.tile([P, DIM], f32, tag="kv")
                      dma(out=_f32r(kv_t), in_=_f32r(kvw_d[c, ko]))
                      for half in range(2):
                          mm(r_ps[half][0:1, :], _f32r(memtokT[:, ko:ko + 1]),
                             _f32r(kv_t[:, half * SEG:(half + 1) * SEG]),
                             start=(ko == 0), stop=(ko == KO - 1))
                  row = mrow[:, DIM:2 * DIM] if c == 0 else mrow[:, 2 * DIM:3 * DIM]
                  for half in range(2):
                      nc.scalar.activation(_f32r(row[:, half * SEG:(half + 1) * SEG]),
                                           r_ps[half][0:1, :], AF.Copy)
                  kcvc.append(row)
              kc_row, vc_row = kcvc
              # k_extra^T into kP (16 identical columns per head)
              for j in range(KO):  # 2 heads per chunk
                  kx = pa_t()
                  mm(kx[:, 0:16], kc_row[0:1, j * P:(j + 1) * P],
                     ones16, start=True, stop=True, skip_group_check=True)
                  nc.vector.tensor_copy(out=_f32r(kP[0:DH, 2 * j, 0:NPM]),
                                        in_=kx[0:DH, 0:16])
                  nc.vector.tensor_copy(out=_f32r(kP[DH:P, 2 * j + 1, 0:NPM]),
                                        in_=kx[DH:P, 0:16])
              # v_extra rows into vP (16 identical rows per head)
              for half in range(2):
                  vx = pa_t()
                  mm(vx[0:16, :], ones16,
                     vc_row[0:1, half * SEG:(half + 1) * SEG],
                     start=True, stop=True, skip_group_check=True)
                  nc.vector.tensor_copy(
                      out=_f32r(vP[0:NPM, half * 8:(half + 1) * 8, :]),
                      in_=vx[0:16, :].rearrange("p (h d) -> p h d", d=DH))

              # ---------------- attention heads ----------------
              for h in range(HEADS):
                  ko_h, hf = h // 2, h % 2
                  qr = DH * hf
                  q_h = qT[qr:qr + DH, ko_h, :]
                  k_h = kT[qr:qr + DH, ko_h, :]
                  # prefix scores [32, 512]
                  scp = psc_t()
                  mm(scp[0:PFX, :], _f32r(kP[qr:qr + DH, h, :]), _f32r(q_h),
                     start=True, stop=True, skip_group_check=True)
                  eP = epool.tile([PFX, SEG], f32, tag="eP")
                  nc.scalar.activation(_f32r(eP), scp[0:PFX, :], AF.Exp,
                                       bias=zeroc[0:PFX])
                  eS = []
                  for c in range(4):
                      w = SEG - P * c
                      sc = psc_t()
                      mm(sc[:, 0:w], _f32r(k_h[:, c * P:(c + 1) * P]),
                         _f32r(q_h[:, c * P:]),
                         start=True, stop=True, skip_group_check=True)
                      et = epool.tile([P, w], f32, tag=("e0" if c < 2 else "e2"))
                      nc.scalar.activation(_f32r(et), sc[:, 0:w], AF.Exp,
                                           bias=zeroc)
                      nc.vector.tensor_mul(_f32r(et[:, 0:P]), et[:, 0:P], maskD)
                      eS.append(et)
                  # P@V and softmax denominator in separate base-0 PSUM tiles
                  po = pso_t()
                  sm = pso_t()
                  mm(po[0:DH, :], _f32r(vP[:, h, :]), _f32r(eP),
                     start=True, stop=False, skip_group_check=True)
                  mm(sm[0:DH, :], _f32r(onesPV[0:PFX, 0:DH]), _f32r(eP),
                     start=True, stop=False, skip_group_check=True)
                  for c in range(4):
                      w = SEG - P * c
                      last = c == 3
                      mm(po[0:DH, c * P:], _f32r(vA[:, c, h, :]), _f32r(eS[c]),
                         start=False, stop=last, skip_group_check=True)
                      mm(sm[0:DH, c * P:], _f32r(onesPV[:, 0:DH]), _f32r(eS[c]),
                         start=False, stop=last, skip_group_check=True)
                  rv = rot.tile([P, SEG], f32, tag="ropeB")
                  nc.vector.reciprocal(rv[0:DH, :], sm[0:DH, :])
                  nc.vector.tensor_mul(_f32r(oA[qr:qr + DH, ko_h, :]),
                                       po[0:DH, :], rv[0:DH, :])

              # ---------------- output projection + residual ----------------
              for m in range(KO):
                  wt = wpool.tile([P, KO, P], f32, tag="w8")
                  dma(out=_f32r(wt), in_=_f32r(outw_d[m]))
                  ps = pa_t()
                  for k in range(KO):
                      mm(ps, _f32r(wt[:, k]), _f32r(oA[:, k, :]),
                         start=(k == 0), stop=(k == KO - 1))
                  nc.vector.tensor_add(xT[:, m, :], ps, xT[:, m, :])  # x1, in place

              # ---------------- FFN ----------------
              rmsnorm_into(xnT, xT, fnw, "big16")  # xn1^T
              u_parts = [qT, kT]  # reuse dead slots as u storage
              u_c = persist.tile([P, 6, SEG], f32, tag="big16")

              def u_slice(k):
                  if k < 8:
                      return u_parts[0][:, k, :]
                  if k < 16:
                      return u_parts[1][:, k - 8, :]
                  return u_c[:, k - 16, :]

              for m in range(MFF):
                  wa = wpool.tile([P, KO, P], f32, tag="w8")
                  dma(out=_f32r(wa), in_=_f32r(w1a_d[m]))
                  wg = wpool.tile([P, KO, P], f32, tag="w8")
                  dma(out=_f32r(wg), in_=_f32r(w1g_d[m]))
                  psa = pa_t()
                  psg = pa_t()
                  for ko in range(KO):
                      mm(psa, _f32r(wa[:, ko]), _f32r(xnT[:, ko, :]),
                         start=(ko == 0), stop=(ko == KO - 1))
                      mm(psg, _f32r(wg[:, ko]), _f32r(xnT[:, ko, :]),
                         start=(ko == 0), stop=(ko == KO - 1))
                  sig = rot.tile([P, SEG], f32, tag="ropeA")
                  nc.scalar.activation(sig, psg, AF.Sigmoid,
                                       bias=b1g[:, m:m + 1], scale=1.0)
                  silu = rot.tile([P, SEG], f32, tag="ropeB")
                  nc.vector.scalar_tensor_tensor(
                      out=silu, in0=psg, scalar=b1g[:, m:m + 1],
                      in1=sig, op0=OP.add, op1=OP.mult)
                  nc.vector.scalar_tensor_tensor(
                      out=_f32r(u_slice(m)), in0=psa, scalar=b1a[:, m:m + 1],
                      in1=silu, op0=OP.add, op1=OP.mult)

              for o in range(KO):
                  ps = pa_t()
                  for half in range(2):
                      w2t = w2pool.tile([P, 11, P], f32, tag="w2")
                      dma(out=_f32r(w2t), in_=_f32r(w2_d[o][:, half * 11:(half + 1) * 11, :]))
                      for k2 in range(11):
                          k = half * 11 + k2
                          mm(ps, _f32r(w2t[:, k2]), _f32r(u_slice(k)),
                             start=(k == 0), stop=(k == MFF - 1))
                  outT = persist.tile([P, KO, SEG], f32, tag="vA",
                                      name=f"outT{o}")
                  nc.vector.scalar_tensor_tensor(
                      out=outT[:, o, :], in0=ps, scalar=b2[:, o:o + 1],
                      in1=xT[:, o, :], op0=OP.add, op1=OP.add)
                  dma(out=yT_d[o * P:(o + 1) * P, :], in_=outT[:, o, :])

    for _rep in range(reps):
        _emit(nc)
    nc.compile()
    return nc


# ======================= host-side preparation =======================

def _prep_shared(inputs):
    f32 = np.float32
    qkv = np.asarray(inputs["to_qkv_w"], f32)
    shared = {}
    shared["qkw"] = np.ascontiguousarray(
        qkv[:, :2048].reshape(KO, P, 16, P).transpose(2, 1, 0, 3))
    shared["kvw"] = np.ascontiguousarray(
        np.stack([qkv[:, 1024:2048], qkv[:, 2048:3072]])
        .reshape(2, KO, P, DIM))
    shared["outw"] = np.ascontiguousarray(
        np.asarray(inputs["to_out_w"], f32)
        .reshape(KO, P, KO, P).transpose(2, 1, 0, 3))
    w1 = np.asarray(inputs["ff_w1"], f32)
    w1a = np.zeros((DIM, DFFP), f32)
    w1g = np.zeros((DIM, DFFP), f32)
    w1a[:, :DFF] = w1[:, :DFF]
    w1g[:, :DFF] = w1[:, DFF:]
    shared["w1a"] = np.ascontiguousarray(
        w1a.reshape(KO, P, MFF, P).transpose(2, 1, 0, 3))
    shared["w1g"] = np.ascontiguousarray(
        w1g.reshape(KO, P, MFF, P).transpose(2, 1, 0, 3))
    w2 = np.zeros((DFFP, DIM), f32)
    w2[:DFF] = np.asarray(inputs["ff_w2"], f32)
    shared["w2"] = np.ascontiguousarray(
        w2.reshape(MFF, P, KO, P).transpose(2, 1, 0, 3))
    shared["mtw"] = np.ascontiguousarray(
        np.asarray(inputs["to_mem_tokens_w"], f32).reshape(KO, P, DIM))
    pm = np.asarray(inputs["persist_mem"], f32)
    shared["pmv"] = np.ascontiguousarray(pm)
    shared["pmk"] = np.ascontiguousarray(pm.transpose(0, 2, 1))
    b1 = np.asarray(inputs["ff_b1"], f32)
    b1a = np.zeros(DFFP, f32)
    b1g = np.zeros(DFFP, f32)
    b1a[:DFF] = b1[:DFF]
    b1g[:DFF] = b1[DFF:]
    shared["b1a"] = np.ascontiguousarray(b1a.reshape(MFF, P).T)
    shared["b1g"] = np.ascontiguousarray(b1g.reshape(MFF, P).T)
    shared["b2"] = np.ascontiguousarray(
        np.asarray(inputs["ff_b2"], f32).reshape(KO, P).T)
    shared["anw"] = np.ascontiguousarray(
        np.asarray(inputs["attn_norm_w"], f32).reshape(KO, P).T)
    shared["fnw"] = np.ascontiguousarray(
        np.asarray(inputs["ff_norm_w"], f32).reshape(KO, P).T)
    shared["mpnw"] = np.ascontiguousarray(
        np.asarray(inputs["mem_pool_norm_w"], f32).reshape(1, DIM))
    rl = np.zeros((P, P), f32)
    ii = np.arange(0, P, 2)
    rl[ii + 1, ii] = f32(-1.0)
    rl[ii, ii + 1] = f32(1.0)
    shared["rmat"] = rl
    shared["ones"] = np.ones((P, P), f32)
    shared["maskD"] = np.where(
        np.arange(P)[None, :] >= np.arange(P)[:, None], f32(1.0), f32(0.0)
    ).astype(f32)

    # rope tables, float32 math to match the reference
    pos = np.arange(N, dtype=f32)
    expo = (np.arange(0, DH, 2).astype(f32) / f32(DH)).astype(f32)
    inv = (f32(1.0) / np.power(f32(10000.0), expo)).astype(f32)
    ang = np.repeat(pos[:, None] * inv[None, :], 2, axis=1).astype(f32)
    cosf, sinf = np.cos(ang).astype(f32), np.sin(ang).astype(f32)
    scale = f32(DH ** -0.5)
    shared["_cos"], shared["_sin"], shared["_scale"] = cosf, sinf, scale
    return shared


def _prep_core(inputs, shared, b, s):
    f32 = np.float32
    x = np.asarray(inputs["x"], f32)
    mo = np.asarray(inputs["mem_out"], f32)
    cosf, sinf, scale = shared["_cos"], shared["_sin"], shared["_scale"]
    seg = slice(s * SEG, (s + 1) * SEG)
    ct = np.ascontiguousarray(np.tile(cosf[seg].T, (2, 1)))
    st = np.ascontiguousarray(np.tile(sinf[seg].T, (2, 1)))
    m = {k: v for k, v in shared.items() if not k.startswith("_")}
    m["xT"] = np.ascontiguousarray(x[b, seg].T)
    m["mo"] = np.ascontiguousarray(mo[b])
    m["cq"] = (ct * scale).astype(f32)
    m["sq"] = (st * scale).astype(f32)
    m["ck"] = ct
    m["sk"] = st
    return m


def _get_nc():
    if "nc" not in _CACHE:
        _CACHE["nc"] = build_nc()
    return _CACHE["nc"]


def kernel(**inputs) -> np.ndarray:
    nc = _get_nc()
    shared = _prep_shared(inputs)
    cores = [(b, s) for b in range(B) for s in range(4)]
    in_maps = [_prep_core(inputs, shared, b, s) for b, s in cores]
    from concourse import bass_utils
    import os
    res = bass_utils.run_bass_kernel_spmd(
        nc, in_maps, core_ids=list(range(NCORES)),
        trace=bool(os.environ.get("MAC_TRACE")))
    _CACHE["last_results"] = res
    out = np.empty((B, N, DIM), np.float32)
    for i, (b, s) in enumerate(cores):
        out[b, s * SEG:(s + 1) * SEG, :] = res.results[i]["yT"].T
    return out



# revision 10
# speedup vs baseline: 1.6830x; 1.6830x over previous
"""Trainium2 Bass kernel for nn_MACBlock (segmented attention + GEGLU FFN).

Sharding: 8 cores = 2 batches x 4 segments of 512 queries. The segment mask
makes attention block-diagonal (plus a 32-token always-visible prefix derived
from pooled memory + persistent memory), so each core is fully independent:
no collectives.

Layout: activations are kept feature-major (x^T [dim, tokens]) on-chip, so
every matmul contraction dim lands on partitions with zero transposes.
All matmuls run with bf16 operands (fp32 PSUM accumulation); weights are
shipped bf16 in partition-major DRAM layouts so each dma_start moves large
per-partition-contiguous lines. The softmax denominator comes from a column
of ones appended to V (one extra PSUM row), and normalization is a bf16
reciprocal of that row broadcast through the PE.
"""

import sys

if "/opt/trn_rl_repo" not in sys.path:
    sys.path.insert(0, "/opt/trn_rl_repo")

import numpy as np

B, N, DIM = 2, 2048, 1024
HEADS, DH = 16, 64
DHP = DH + 1            # +1 denominator column
SEG = 512
NPM = NM = 16
PFX = NPM + NM          # 32 prefix keys
DFF = 2730
MFF = 22                # padded dff chunks
DFFP = MFF * 128        # 2816
KO = 8                  # 1024 / 128
P = 128
NCORES = 8
EPS = 1.1920929e-07

_CACHE = {}


def build_nc(reps=1):
    import concourse.bass as bass
    from concourse import bacc
    import concourse.tile as tile
    import concourse.mybir as mybir

    f32 = mybir.dt.float32
    bf = mybir.dt.bfloat16
    AF = mybir.ActivationFunctionType
    OP = mybir.AluOpType
    AX = mybir.AxisListType

    nc = bacc.Bacc("TRN2", target_bir_lowering=False, debug=False)

    dp = nc.declare_dram_parameter
    xT_d = dp("xT", [P, KO, SEG], f32, isOutput=False)
    mo_d = dp("mo", [P, 16, DIM], bf, isOutput=False)
    cq_d = dp("cq", [P, SEG], f32, isOutput=False)
    sq_d = dp("sq", [P, SEG], f32, isOutput=False)
    ck_d = dp("ck", [P, SEG], f32, isOutput=False)
    sk_d = dp("sk", [P, SEG], f32, isOutput=False)
    mask_d = dp("maskD", [P, P], bf, isOutput=False)
    rmat_d = dp("rmat", [P, P], bf, isOutput=False)
    ones_d = dp("ones", [P, P], bf, isOutput=False)
    qkw_d = dp("qkw", [P, 16, KO, P], bf, isOutput=False)
    kvw_d = dp("kvw", [P, 2, KO, DIM], bf, isOutput=False)
    outw_d = dp("outw", [P, KO, KO, P], bf, isOutput=False)
    w1_d = dp("w1", [P, MFF, 2, KO, P], bf, isOutput=False)
    w2_d = dp("w2", [KO, P, MFF, P], bf, isOutput=False)
    mtw_d = dp("mtw", [P, KO, DIM], bf, isOutput=False)
    pmv_d = dp("pmv", [HEADS, NPM, DHP], bf, isOutput=False)
    pmk_d = dp("pmk", [HEADS, DH, NPM], bf, isOutput=False)
    b1a_d = dp("b1a", [P, MFF], f32, isOutput=False)
    b1g_d = dp("b1g", [P, MFF], f32, isOutput=False)
    b2_d = dp("b2", [P, KO], f32, isOutput=False)
    anw_d = dp("anw", [P, KO], f32, isOutput=False)
    fnw_d = dp("fnw", [P, KO], f32, isOutput=False)
    mpnw_d = dp("mpnw", [1, DIM], f32, isOutput=False)
    yT_d = dp("yT", [DIM, SEG], f32, isOutput=True)

    def _emit(nc):
      with tile.TileContext(nc) as tc, \
            nc.allow_low_precision(reason="bf16 matmul datapath"):
        from contextlib import ExitStack
        ctx = ExitStack()
        with ctx:
            persist = ctx.enter_context(tc.tile_pool(name="persist", bufs=1))
            wpool = ctx.enter_context(tc.tile_pool(name="wpool", bufs=3))
            kvres = ctx.enter_context(tc.tile_pool(name="kvres", bufs=1))
            w2pool = ctx.enter_context(tc.tile_pool(name="w2pool", bufs=2))
            mopool = ctx.enter_context(tc.tile_pool(name="mopool", bufs=2))
            rot = ctx.enter_context(tc.tile_pool(name="rot", bufs=2))
            epool = ctx.enter_context(tc.tile_pool(name="epool", bufs=2))
            pa = ctx.enter_context(tc.tile_pool(name="pa", bufs=4, space="PSUM"))
            psc = ctx.enter_context(tc.tile_pool(name="psc", bufs=2, space="PSUM"))
            pso = ctx.enter_context(tc.tile_pool(name="pso", bufs=2, space="PSUM"))

            cnt = [0]

            def pa_t():
                cnt[0] += 1
                return pa.tile([P, SEG], f32, tag="ps", name=f"pa{cnt[0]}")

            def psc_t():
                cnt[0] += 1
                return psc.tile([P, SEG], f32, tag="sc", name=f"sc{cnt[0]}")

            def pso_t():
                cnt[0] += 1
                return pso.tile([P, SEG], f32, tag="o", name=f"o{cnt[0]}")

            # ---------------- persistent SBUF tensors ----------------
            xT = persist.tile([P, KO, SEG], f32, tag="xT")       # x^T, later x1^T
            xnT = persist.tile([P, KO, SEG], bf, tag="xnT")      # xn^T (bf16)
            qT = persist.tile([P, KO, SEG], bf, tag="qT")        # roped,scaled q^T
            kT = persist.tile([P, KO, SEG], bf, tag="kT")        # roped k^T
            vA = persist.tile([P, 4, HEADS, DHP], bf, tag="vA")  # v key-major +ones
            vP = persist.tile([PFX, HEADS, DHP], bf, tag="vP")   # prefix v rows +ones
            kP = persist.tile([P, HEADS, PFX], bf, tag="kP")     # prefix k^T @64*(h%2)
            oA = persist.tile([P, KO, SEG], bf, tag="oA")        # attn o^T
            cq = persist.tile([P, SEG], f32, tag="cq")
            sq_ = persist.tile([P, SEG], f32, tag="sq")
            ck = persist.tile([P, SEG], f32, tag="ck")
            sk = persist.tile([P, SEG], f32, tag="sk")
            maskD = persist.tile([P, P], bf, tag="maskD")
            rmat = persist.tile([P, P], bf, tag="rmat")
            b1a = persist.tile([P, MFF], f32, tag="b1a")
            b1g = persist.tile([P, MFF], f32, tag="b1g")
            b2 = persist.tile([P, KO], f32, tag="b2")
            anw = persist.tile([P, KO], f32, tag="anw")
            fnw = persist.tile([P, KO], f32, tag="fnw")
            mpnw = persist.tile([1, DIM], f32, tag="mpnw")
            ones128 = persist.tile([P, 1], bf, tag="o128")       # lhsT K=128,M=1
            ones1x128 = persist.tile([1, P], bf, tag="o1x128")   # lhsT K=1,M=128
            ones16 = persist.tile([1, 16], bf, tag="o16")
            ones11 = persist.tile([1, 1], bf, tag="o11")
            pooledT = persist.tile([P, KO], bf, tag="pooledT")
            memtokT = persist.tile([P, KO], bf, tag="memtokT")
            mrow = persist.tile([1, 2 * DIM], f32, tag="mrow")
            brow = persist.tile([1, 2 * DIM], bf, tag="brow")    # bf16 row scratch
            rrow = persist.tile([1, SEG], bf, tag="rrow")        # bf16 rstd row
            u_c = persist.tile([P, 6, SEG], bf, tag="u_c")       # ffn u chunks 16-21
            epsc = persist.tile([P, 1], f32, tag="epsc")
            zeroc = persist.tile([P, 1], f32, tag="zeroc")

            dma = nc.sync.dma_start
            dma(out=cq, in_=cq_d[:])
            dma(out=sq_, in_=sq_d[:])
            dma(out=ck, in_=ck_d[:])
            dma(out=sk, in_=sk_d[:])
            dma(out=maskD, in_=mask_d[:])
            dma(out=rmat, in_=rmat_d[:])
            dma(out=b1a, in_=b1a_d[:])
            dma(out=b1g, in_=b1g_d[:])
            dma(out=b2, in_=b2_d[:])
            dma(out=anw, in_=anw_d[:])
            dma(out=fnw, in_=fnw_d[:])
            dma(out=mpnw, in_=mpnw_d[:])
            dma(out=ones128, in_=ones_d[:, 0:1])
            dma(out=ones1x128, in_=ones_d[0:1, :])
            dma(out=ones16, in_=ones_d[0:1, 0:16])
            dma(out=ones11, in_=ones_d[0:1, 0:1])
            nc.vector.memset(epsc, EPS)
            nc.vector.memset(zeroc, 0.0)
            nc.vector.memset(vA[:, :, :, DH:DHP], 1.0)
            nc.vector.memset(vP[:, :, DH:DHP], 1.0)
            for h in range(HEADS):
                hb = DH * (h % 2)
                dma(out=kP[hb:hb + DH, h, NPM:PFX], in_=pmk_d[h])
                dma(out=vP[NPM:PFX, h, :], in_=pmv_d[h])

            if True:
              dma(out=xT, in_=xT_d[:])
              kvw = kvres.tile([P, 2, KO, DIM], bf, tag="kv")
              dma(out=kvw, in_=kvw_d[:])

              mm = nc.tensor.matmul

              def rmsnorm_into(dst, src, w_sb):
                  """dst[:,ko,:] = src[:,ko,:] * w[:,ko] * rsqrt(mean_dim(src^2)+eps)"""
                  ss = psc_t()  # [1,512] slice used
                  for ko in range(KO):
                      sq_t = rot.tile([P, SEG], bf, tag="sqt")
                      nc.vector.tensor_mul(sq_t, src[:, ko, :],
                                           src[:, ko, :])
                      mm(ss[0:1, :], ones128, sq_t,
                         start=(ko == 0), stop=(ko == KO - 1))
                  nc.scalar.activation(mrow[0:1, 0:SEG], ss[0:1, :], AF.Sqrt,
                                       bias=epsc[0:1], scale=1.0 / DIM)
                  nc.vector.reciprocal(rrow, mrow[0:1, 0:SEG])
                  bc = pso_t()  # broadcast rstd over 128 partitions
                  mm(bc, ones1x128, rrow, start=True, stop=True)
                  for ko in range(KO):
                      nc.vector.scalar_tensor_tensor(
                          out=dst[:, ko, :], in0=src[:, ko, :],
                          scalar=w_sb[:, ko:ko + 1], in1=bc,
                          op0=OP.mult, op1=OP.mult)

              # ---------------- attn rmsnorm ----------------
              rmsnorm_into(xnT, xT, anw)

              # ---------------- q/k projections + rope, interleaved with
              # ---------------- mem_out mean accumulation ----------------
              mean_ps = [psc_t(), psc_t()]   # two [1,512] accumulators (slices)

              for m in range(16):
                  if m % 4 == 0:
                      wt = wpool.tile([P, 4, KO, P], bf, tag="w")
                      dma(out=wt, in_=qkw_d[:, m:m + 4])
                  if m % 2 == 0:
                      mot = mopool.tile([P, 2, DIM], bf, tag="mo")
                      dma(out=mot, in_=mo_d[:, m:m + 2])
                  ps = pa_t()
                  for ko in range(KO):
                      mm(ps, wt[:, m % 4, ko], xnT[:, ko, :],
                         start=(ko == 0), stop=(ko == KO - 1))
                  is_q = m < 8
                  c_t, s_t = (cq, sq_) if is_q else (ck, sk)
                  dst = qT if is_q else kT
                  ko_out = m % 8
                  qraw = rot.tile([P, SEG], bf, tag="ropeA")
                  nc.scalar.copy(qraw, ps)
                  rps = pa_t()
                  mm(rps, rmat, qraw, start=True, stop=True)
                  At = rot.tile([P, SEG], bf, tag="ropeB")
                  nc.vector.tensor_mul(At, ps, c_t)
                  Bt = rot.tile([P, SEG], bf, tag="ropeC")
                  nc.vector.tensor_mul(Bt, rps, s_t)
                  nc.vector.tensor_add(dst[:, ko_out, :], At, Bt)
                  # mem_out mean accumulation (chunk m)
                  for half in range(2):
                      mm(mean_ps[half][0:1, :], ones128,
                         mot[:, m % 2, half * SEG:(half + 1) * SEG],
                         start=(m == 0), stop=(m == 15))

              # ---------------- v projection (token-major) ----------------
              for half in range(2):
                  for tc_ in range(4):
                      ps = pa_t()
                      for ko in range(KO):
                          mm(ps, xnT[:, ko, tc_ * P:(tc_ + 1) * P],
                             kvw[:, 1, ko, half * SEG:(half + 1) * SEG],
                             start=(ko == 0), stop=(ko == KO - 1))
                      nc.vector.tensor_copy(
                          out=vA[:, tc_, half * 8:(half + 1) * 8, 0:DH],
                          in_=ps.rearrange("p (h d) -> p h d", d=DH))

              # ---------------- memory-context chain ----------------
              pooled_raw = mrow[:, 0:DIM]
              for half in range(2):
                  nc.scalar.activation(pooled_raw[:, half * SEG:(half + 1) * SEG],
                                       mean_ps[half][0:1, :], AF.Copy,
                                       scale=1.0 / N)
              sqr = mrow[:, DIM:2 * DIM]
              nc.vector.tensor_mul(sqr, pooled_raw, pooled_raw)
              nc.vector.reduce_sum(sqr[:, 0:1], sqr, axis=AX.X)
              nc.scalar.activation(sqr[:, 1:2], sqr[:, 0:1], AF.Sqrt,
                                   bias=epsc[0:1], scale=1.0 / DIM)
              nc.vector.reciprocal(sqr[:, 2:3], sqr[:, 1:2])
              pooled = brow[:, 0:DIM]
              nc.vector.scalar_tensor_tensor(out=pooled, in0=pooled_raw,
                                             scalar=sqr[:, 2:3], in1=mpnw,
                                             op0=OP.mult, op1=OP.mult)
              # pooled^T via K=1 transpose matmuls
              pT = pa_t()
              for ko in range(KO):
                  mm(pT[:, ko:ko + 1], pooled[0:1, ko * P:(ko + 1) * P],
                     ones11, start=True, stop=True, skip_group_check=True)
              nc.vector.tensor_copy(out=pooledT, in_=pT[:, 0:KO])
              # mem_tok row = pooled @ to_mem_tokens_w
              mt_ps = [psc_t(), psc_t()]
              for j in range(2):
                  mtw_t = w2pool.tile([P, 4, DIM], bf, tag="mtw")
                  dma(out=mtw_t, in_=mtw_d[:, 4 * j:4 * j + 4])
                  for k2 in range(4):
                      ko = 4 * j + k2
                      for half in range(2):
                          mm(mt_ps[half][0:1, :], pooledT[:, ko:ko + 1],
                             mtw_t[:, k2, half * SEG:(half + 1) * SEG],
                             start=(ko == 0), stop=(ko == KO - 1))
              memtok = brow[:, DIM:2 * DIM]
              for half in range(2):
                  nc.scalar.activation(memtok[:, half * SEG:(half + 1) * SEG],
                                       mt_ps[half][0:1, :], AF.Copy)
              mT = pa_t()
              for ko in range(KO):
                  mm(mT[:, ko:ko + 1], memtok[0:1, ko * P:(ko + 1) * P],
                     ones11, start=True, stop=True, skip_group_check=True)
              nc.vector.tensor_copy(out=memtokT, in_=mT[:, 0:KO])
              # k_c / v_c rows = mem_tok @ Wk / Wv
              kcvc = []
              for c in range(2):
                  r_ps = [psc_t(), psc_t()]
                  for ko in range(KO):
                      for half in range(2):
                          mm(r_ps[half][0:1, :], memtokT[:, ko:ko + 1],
                             kvw[:, c, ko, half * SEG:(half + 1) * SEG],
                             start=(ko == 0), stop=(ko == KO - 1))
                  row = brow[:, 0:DIM] if c == 0 else brow[:, DIM:2 * DIM]
                  for half in range(2):
                      nc.scalar.activation(row[:, half * SEG:(half + 1) * SEG],
                                           r_ps[half][0:1, :], AF.Copy)
                  kcvc.append(row)
              kc_row, vc_row = kcvc
              # k_extra^T into kP (16 identical columns per head)
              for j in range(KO):  # 2 heads per chunk
                  kx = pa_t()
                  mm(kx[:, 0:16], kc_row[0:1, j * P:(j + 1) * P],
                     ones16, start=True, stop=True, skip_group_check=True)
                  nc.vector.tensor_copy(out=kP[0:DH, 2 * j, 0:NPM],
                                        in_=kx[0:DH, 0:16])
                  nc.vector.tensor_copy(out=kP[DH:P, 2 * j + 1, 0:NPM],
                                        in_=kx[DH:P, 0:16])
              # v_extra rows into vP (16 identical rows per head)
              for half in range(2):
                  vx = pa_t()
                  mm(vx[0:16, :], ones16,
                     vc_row[0:1, half * SEG:(half + 1) * SEG],
                     start=True, stop=True, skip_group_check=True)
                  nc.vector.tensor_copy(
                      out=vP[0:NPM, half * 8:(half + 1) * 8, 0:DH],
                      in_=vx[0:16, :].rearrange("p (h d) -> p h d", d=DH))

              # ---------------- attention heads ----------------
              for h in range(HEADS):
                  ko_h, hf = h // 2, h % 2
                  qr = DH * hf
                  q_h = qT[qr:qr + DH, ko_h, :]
                  k_h = kT[qr:qr + DH, ko_h, :]
                  # prefix scores [32, 512]
                  scp = psc_t()
                  mm(scp[0:PFX, :], kP[qr:qr + DH, h, :], q_h,
                     start=True, stop=True, skip_group_check=True)
                  eP = epool.tile([PFX, SEG], bf, tag="eP")
                  nc.scalar.activation(eP, scp[0:PFX, :], AF.Exp,
                                       bias=zeroc[0:PFX])
                  eS = []
                  for c in range(4):
                      w = SEG - P * c
                      sc = psc_t()
                      mm(sc[:, 0:w], k_h[:, c * P:(c + 1) * P],
                         q_h[:, c * P:],
                         start=True, stop=True, skip_group_check=True)
                      et = epool.tile([P, w], bf, tag=("e0" if c < 2 else "e2"))
                      nc.scalar.activation(et, sc[:, 0:w], AF.Exp,
                                           bias=zeroc)
                      nc.vector.tensor_mul(et[:, 0:P], et[:, 0:P], maskD)
                      eS.append(et)
                  # P@V with ones column: row DH of po = softmax denominator
                  po = pso_t()
                  mm(po[0:DHP, :], vP[:, h, :], eP,
                     start=True, stop=False, skip_group_check=True)
                  for c in range(4):
                      w = SEG - P * c
                      mm(po[0:DHP, c * P:], vA[:, c, h, :], eS[c],
                         start=False, stop=(c == 3), skip_group_check=True)
                  dnr = rot.tile([1, SEG], bf, tag="dnr")
                  nc.vector.reciprocal(dnr, po[DH:DHP, :])
                  dnb = rot.tile([DH, SEG], bf, tag="dnb")
                  nc.gpsimd.partition_broadcast(dnb, dnr[0:1, :], channels=DH)
                  nc.vector.tensor_mul(oA[qr:qr + DH, ko_h, :],
                                       po[0:DH, :], dnb)

              # ---------------- output projection + residual ----------------
              for m in range(KO):
                  if m % 4 == 0:
                      owt = wpool.tile([P, 4, KO, P], bf, tag="w")
                      dma(out=owt, in_=outw_d[:, m:m + 4])
                  ps = pa_t()
                  for k in range(KO):
                      mm(ps, owt[:, m % 4, k], oA[:, k, :],
                         start=(k == 0), stop=(k == KO - 1))
                  nc.vector.tensor_add(xT[:, m, :], ps, xT[:, m, :])  # x1

              # ---------------- FFN ----------------
              rmsnorm_into(xnT, xT, fnw)  # xn1^T
              u_parts = [qT, kT]  # reuse dead slots as u storage

              def u_slice(k):
                  if k < 8:
                      return u_parts[0][:, k, :]
                  if k < 16:
                      return u_parts[1][:, k - 8, :]
                  return u_c[:, k - 16, :]

              for j in range(11):
                  wt1 = wpool.tile([P, 2, 2, KO, P], bf, tag="w")
                  dma(out=wt1, in_=w1_d[:, 2 * j:2 * j + 2])
                  for i in range(2):
                      m = 2 * j + i
                      psa = pa_t()
                      psg = pa_t()
                      for ko in range(KO):
                          mm(psa, wt1[:, i, 0, ko], xnT[:, ko, :],
                             start=(ko == 0), stop=(ko == KO - 1))
                          mm(psg, wt1[:, i, 1, ko], xnT[:, ko, :],
                             start=(ko == 0), stop=(ko == KO - 1))
                      silu = rot.tile([P, SEG], bf, tag="silu")
                      nc.scalar.activation(silu, psg, AF.Silu,
                                           bias=b1g[:, m:m + 1], scale=1.0)
                      nc.vector.scalar_tensor_tensor(
                          out=u_slice(m), in0=psa, scalar=b1a[:, m:m + 1],
                          in1=silu, op0=OP.add, op1=OP.mult)

              for o in range(KO):
                  w2t = w2pool.tile([P, MFF, P], bf, tag="w2")
                  dma(out=w2t, in_=w2_d[o])
                  ps = pa_t()
                  for k in range(MFF):
                      mm(ps, w2t[:, k], u_slice(k),
                         start=(k == 0), stop=(k == MFF - 1))
                  outT = rot.tile([P, SEG], f32, tag="outT")
                  nc.vector.scalar_tensor_tensor(
                      out=outT, in0=ps, scalar=b2[:, o:o + 1],
                      in1=xT[:, o, :], op0=OP.add, op1=OP.add)
                  dma(out=yT_d[o * P:(o + 1) * P, :], in_=outT)

    for _rep in range(reps):
        _emit(nc)
    nc.compile()
    return nc


# ======================= host-side preparation =======================

def _prep_shared(inputs):
    import ml_dtypes
    f32 = np.float32
    bf = ml_dtypes.bfloat16
    qkv = np.asarray(inputs["to_qkv_w"], f32)
    shared = {}
    # q/k projection weights: [p_in, m, ko, p_out]
    shared["qkw"] = np.ascontiguousarray(
        qkv[:, :2048].reshape(KO, P, 16, P).transpose(1, 2, 0, 3)).astype(bf)
    # k/v full weights: [p_in, c, ko, out]
    shared["kvw"] = np.ascontiguousarray(
        np.stack([qkv[:, 1024:2048], qkv[:, 2048:3072]])
        .reshape(2, KO, P, DIM).transpose(2, 0, 1, 3)).astype(bf)
    # out projection: [p_in, m, k, p_out]
    shared["outw"] = np.ascontiguousarray(
        np.asarray(inputs["to_out_w"], f32)
        .reshape(KO, P, KO, P).transpose(1, 2, 0, 3)).astype(bf)
    w1 = np.asarray(inputs["ff_w1"], f32)
    w1a = np.zeros((DIM, DFFP), f32)
    w1g = np.zeros((DIM, DFFP), f32)
    w1a[:, :DFF] = w1[:, :DFF]
    w1g[:, :DFF] = w1[:, DFF:]
    # ffn w1: [p_in, m, s(a/g), ko, p_out]
    shared["w1"] = np.ascontiguousarray(
        np.stack([w1a, w1g]).reshape(2, KO, P, MFF, P)
        .transpose(2, 3, 0, 1, 4)).astype(bf)
    w2 = np.zeros((DFFP, DIM), f32)
    w2[:DFF] = np.asarray(inputs["ff_w2"], f32)
    # ffn w2: [o, p_in, k, p_out]
    shared["w2"] = np.ascontiguousarray(
        w2.reshape(MFF, P, KO, P).transpose(2, 1, 0, 3)).astype(bf)
    # mem tokens w: [p_in, ko, out]
    shared["mtw"] = np.ascontiguousarray(
        np.asarray(inputs["to_mem_tokens_w"], f32)
        .reshape(KO, P, DIM).transpose(1, 0, 2)).astype(bf)
    pm = np.asarray(inputs["persist_mem"], f32)
    pmv = np.ones((HEADS, NPM, DHP), f32)
    pmv[:, :, :DH] = pm
    shared["pmv"] = pmv.astype(bf)
    shared["pmk"] = np.ascontiguousarray(pm.transpose(0, 2, 1)).astype(bf)
    b1 = np.asarray(inputs["ff_b1"], f32)
    b1a = np.zeros(DFFP, f32)
    b1g = np.zeros(DFFP, f32)
    b1a[:DFF] = b1[:DFF]
    b1g[:DFF] = b1[DFF:]
    shared["b1a"] = np.ascontiguousarray(b1a.reshape(MFF, P).T)
    shared["b1g"] = np.ascontiguousarray(b1g.reshape(MFF, P).T)
    shared["b2"] = np.ascontiguousarray(
        np.asarray(inputs["ff_b2"], f32).reshape(KO, P).T)
    shared["anw"] = np.ascontiguousarray(
        np.asarray(inputs["attn_norm_w"], f32).reshape(KO, P).T)
    shared["fnw"] = np.ascontiguousarray(
        np.asarray(inputs["ff_norm_w"], f32).reshape(KO, P).T)
    shared["mpnw"] = np.ascontiguousarray(
        np.asarray(inputs["mem_pool_norm_w"], f32).reshape(1, DIM))
    rl = np.zeros((P, P), f32)
    ii = np.arange(0, P, 2)
    rl[ii + 1, ii] = f32(-1.0)
    rl[ii, ii + 1] = f32(1.0)
    shared["rmat"] = rl.astype(bf)
    shared["ones"] = np.ones((P, P), bf)
    shared["maskD"] = np.where(
        np.arange(P)[None, :] >= np.arange(P)[:, None], f32(1.0), f32(0.0)
    ).astype(bf)

    # rope tables, float32 math to match the reference
    pos = np.arange(N, dtype=f32)
    expo = (np.arange(0, DH, 2).astype(f32) / f32(DH)).astype(f32)
    inv = (f32(1.0) / np.power(f32(10000.0), expo)).astype(f32)
    ang = np.repeat(pos[:, None] * inv[None, :], 2, axis=1).astype(f32)
    cosf, sinf = np.cos(ang).astype(f32), np.sin(ang).astype(f32)
    scale = f32(DH ** -0.5)
    shared["_cos"], shared["_sin"], shared["_scale"] = cosf, sinf, scale
    return shared


def _prep_core(inputs, shared, b, s):
    import ml_dtypes
    f32 = np.float32
    bf = ml_dtypes.bfloat16
    x = np.asarray(inputs["x"], f32)
    mo = np.asarray(inputs["mem_out"], f32)
    cosf, sinf, scale = shared["_cos"], shared["_sin"], shared["_scale"]
    seg = slice(s * SEG, (s + 1) * SEG)
    ct = np.ascontiguousarray(np.tile(cosf[seg].T, (2, 1)))
    st = np.ascontiguousarray(np.tile(sinf[seg].T, (2, 1)))
    m = {k: v for k, v in shared.items() if not k.startswith("_")}
    m["xT"] = np.ascontiguousarray(
        x[b, seg].T.reshape(KO, P, SEG).transpose(1, 0, 2))
    m["mo"] = np.ascontiguousarray(
        mo[b].reshape(16, P, DIM).transpose(1, 0, 2)).astype(bf)
    m["cq"] = (ct * scale).astype(f32)
    m["sq"] = (st * scale).astype(f32)
    m["ck"] = ct
    m["sk"] = st
    return m


def _get_nc():
    if "nc" not in _CACHE:
        _CACHE["nc"] = build_nc()
    return _CACHE["nc"]


def kernel(**inputs) -> np.ndarray:
    nc = _get_nc()
    shared = _prep_shared(inputs)
    cores = [(b, s) for b in range(B) for s in range(4)]
    in_maps = [_prep_core(inputs, shared, b, s) for b, s in cores]
    from concourse import bass_utils
    import os
    res = bass_utils.run_bass_kernel_spmd(
        nc, in_maps, core_ids=list(range(NCORES)),
        trace=bool(os.environ.get("MAC_TRACE")))
    _CACHE["last_results"] = res
    out = np.empty((B, N, DIM), np.float32)
    for i, (b, s) in enumerate(cores):
        out[b, s * SEG:(s + 1) * SEG, :] = res.results[i]["yT"].T
    return out
